# revision 22
# baseline (speedup 1.0000x reference)
"""Trainium2 Bass kernel for CodeRecursiveNeuralNetworks (tree-RNN over complete
binary trees, heap layout).

Math (per tree, heap order: node i has parent (i-1)//2, level d = [2^d-1, 2^{d+1}-1)):
    x = E[node_type];  h_leaf = tanh(x_leaf)
    for d = 8..0:  h_d = tanh(x_d + (h_{d+1,even} + h_{d+1,odd}) @ Wh + bh)
    logits = h_root @ Wc + bc;  out = log_softmax(logits)

Strategy (8 cores, data-parallel over trees; 32 trees/core, no collectives):
  - Everything in "transposed" layout [H=128 partitions, nodes free]; levels are
    tree-major so children of parent column p are columns 2p, 2p+1.
  - Embedding lookups are one-hot matmuls (V=100 rows + 1 bias row = 101).
    G = tanh(E) @ Wh is precomputed on the HOST, folding the leaf level away:
    level-8 psum = E^T oh(x8) + G^T C8 with C8[:,p] = oh(a_p)+oh(b_p).
  - Level 8 runs as a single fp8 DoubleRow matmul pass: lhsT = [E|G] stacked in
    the doubled contraction dim, rhs = [x8-plane | C8-plane] band pairs. One
    column per pass instead of two.
  - h is stored fp16 DE-INTERLEAVED (even children in the first half, odd in the
    second): pairsums become contiguous tensor_tensor adds on the DVE (fast 2x/4x
    modes), and small levels use two contiguous-half bf16 matmuls.
  - Levels 7..4 pairs: DVE pairsum + one fp16 Wh matmul. Levels 3..0: two
    stride-half matmuls accumulating onto E-prefilled psum (no DVE hop).
  - bias bh rides row 100 of every x one-hot column (E_aug row 100 = bh), so
    tanh needs no bias operand and fuses psum->fp16 SBUF in one ACT instr.
  - log_softmax entirely without a second ACT table: logits computed directly in
    [tree, class] layout (h0 as lhsT), exp uses the already-loaded table, and
    ln(s) for s in [1,6] is a deg-7 Horner polynomial on the DVE.
  - One-hot DMA: x8/C8 planes split across both HWDGE rings band-by-band in PE
    consumption order; weights first, constants via SWDGE.
"""

import numpy as np
import ml_dtypes

B = 256
M = 1023
H = 128
V = 100
KV = V                    # one-hot rows (bias via ACT)
NCLS = 6
CORES = 8
TPC = B // CORES          # trees per core
N8 = TPC * 256            # level-8 cols per core (8192)

LN = {d: TPC * (1 << d) for d in range(9)}   # cols per level, per core

# one-hot flat layout [KV, TOTAL]:
#   [ L8 bands: 8 x (x8 1024 | c8 1024) | x7 4096 | x6 2048 | x5..x0 2016 ]
L8_BAND = 1024
N_L8_BANDS = N8 // L8_BAND                   # 8
OFF_X = {}
_off = 2 * N8                                # 16384
for _d in range(7, -1, -1):
    OFF_X[_d] = _off
    _off += LN[_d]
TOTAL_COLS = _off                            # 24544
OFF_SMALL = OFF_X[5]                         # x5..x0 region start (22528)

# DRAM layout: 12 chunk-contiguous [KV, 2048] blocks (seq HBM reads):
#   0..7 = L8 bands, 8/9 = x7a/x7b, 10 = x6, 11 = x5..x0 (+32 pad)
N_CHUNKS = 12
CHUNK_W = 2048
CHUNK_FLAT = [(2048 * b, 2048) for b in range(8)] + [
    (OFF_X[7], 2048), (OFF_X[7] + 2048, 2048), (OFF_X[6], 2048),
    (OFF_SMALL, 2016)]

# log(s) on s in [1, 6.1], highest-degree first (deg 7)
LN_POLY = [6.128034284263229e-05, -0.0017329973884222805, 0.020805833546868057,
           -0.13863827927501848, 0.5645312741448621, -1.4748635041826688,
           2.701343407709817, -1.6707463490660965]

L8_DR = True              # level 8 via fp8 DoubleRow merged [E|G] matmul

_PROGRAMS = {}


def _build_program(l8_dr=L8_DR, debug_taps=False):
    import concourse.bacc as bacc
    import concourse.tile as tile
    import concourse.mybir as mybir

    dt = mybir.dt
    AF = mybir.ActivationFunctionType
    ALU = mybir.AluOpType
    AX = mybir.AxisListType
    PM = mybir.MatmulPerfMode

    nc = bacc.Bacc("TRN2", target_bir_lowering=False, debug=False)

    oh_d = nc.dram_tensor("oh", [N_CHUNKS, KV, CHUNK_W], dt.float8e4,
                          kind="ExternalInput")
    eaug_d = nc.dram_tensor("e_aug", [KV, H], dt.float16, kind="ExternalInput")
    egdr_d = nc.dram_tensor("eg_dr", [KV, 2 * H], dt.float8e4, kind="ExternalInput")
    gaug_d = nc.dram_tensor("g_aug", [KV, H], dt.float16, kind="ExternalInput")
    wh_d = nc.dram_tensor("wh", [H, H], dt.float16, kind="ExternalInput")
    wc_d = nc.dram_tensor("wc", [H, NCLS], dt.float16, kind="ExternalInput")
    bc_d = nc.dram_tensor("bc_rep", [TPC, NCLS], dt.float32, kind="ExternalInput")
    bh_d = nc.dram_tensor("bh", [H, 1], dt.float32, kind="ExternalInput")
    out_d = nc.dram_tensor("out", [TPC, NCLS], dt.float32, kind="ExternalOutput")

    with tile.TileContext(nc) as tc:
        with (
            tc.tile_pool(name="const", bufs=1) as cpool,
            tc.tile_pool(name="bandp", bufs=12) as bpool,
            tc.tile_pool(name="hp", bufs=1) as hpool,
            tc.tile_pool(name="aggp", bufs=1) as apool,
            tc.tile_pool(name="psp", bufs=2, space="PSUM") as pspool,
            tc.tile_pool(name="smallp", bufs=1) as smpool,
        ):
            # ---- PE warm-up: junk matmuls to open the HAM clock gate ----
            junk = cpool.tile([H, 512], dt.float16, tag="junk")
            nc.gpsimd.memset(junk[:], 0)

            egdr = cpool.tile([KV, 2 * H], dt.float8e4, tag="egdr")
            eaug = cpool.tile([KV, H], dt.float16, tag="eaug")
            gaug = cpool.tile([KV, H], dt.float16, tag="gaug")
            bh_t = cpool.tile([H, 1], dt.float32, tag="bh")
            wh = cpool.tile([H, H], dt.float16, tag="wh")
            wc = cpool.tile([H, NCLS], dt.float16, tag="wc")
            bc_rep = cpool.tile([TPC, NCLS], dt.float32, tag="bc_rep")

            def load_chunk(eng, idx, nm, ncols=CHUNK_W):
                t = bpool.tile([KV, 2048], dt.float8e4, tag="band", name=nm)
                eng.dma_start(out=t[:, :ncols], in_=oh_d[idx, :, :ncols])
                return t

            # L8 bands spread across all 3 queues in PE consumption order.
            # First transfer per queue crawls (~2-3us ramp), so the payload
            # bands lead and the small weights ride behind them.
            # Chunk0's two bands are split in halves across ALL queues so the
            # first psum chunk lands ~12.5us even when one queue is starved
            # by the early fabric arbitration. Remaining bands stripe the
            # queues in consumption order; small weights ride between.
            l8_bands = [None] * N_L8_BANDS

            def half_load(eng, idx, t, lo):
                eng.dma_start(out=t[:, lo:lo + 1024], in_=oh_d[idx, :, lo:lo + 1024])

            b0 = bpool.tile([KV, 2048], dt.float8e4, tag="band", name="b8_0")
            b1 = bpool.tile([KV, 2048], dt.float8e4, tag="band", name="b8_1")
            l8_bands[0], l8_bands[1] = b0, b1
            half_load(nc.sync, 0, b0, 0)
            if l8_dr:
                nc.scalar.dma_start(out=egdr[:], in_=egdr_d[:])
            else:
                nc.scalar.dma_start(out=gaug[:], in_=gaug_d[:])
            nc.gpsimd.dma_start(out=bh_t[:], in_=bh_d[:])
            half_load(nc.sync, 1, b1, 0)
            half_load(nc.scalar, 0, b0, 1024)
            half_load(nc.gpsimd, 1, b1, 1024)

            l8_bands[2] = load_chunk(nc.scalar, 2, "b8_2")
            l8_bands[3] = load_chunk(nc.gpsimd, 3, "b8_3")
            l8_bands[4] = load_chunk(nc.sync, 4, "b8_4")
            l8_bands[5] = load_chunk(nc.scalar, 5, "b8_5")
            l8_bands[6] = load_chunk(nc.gpsimd, 6, "b8_6")
            l8_bands[7] = load_chunk(nc.sync, 7, "b8_7")

            nc.gpsimd.dma_start(out=eaug[:], in_=eaug_d[:])
            x7a = load_chunk(nc.scalar, 8, "x7a")
            nc.gpsimd.dma_start(out=wh[:], in_=wh_d[:])
            x7b = load_chunk(nc.gpsimd, 9, "x7b")
            x6t = load_chunk(nc.scalar, 10, "x6")
            smt = load_chunk(nc.sync, 11, "small", 2016)
            nc.gpsimd.dma_start(out=wc[:], in_=wc_d[:])
            nc.gpsimd.dma_start(out=bc_rep[:], in_=bc_d[:])

            # ---- h tiles (fp16, de-interleaved: evens then odds) ----
            h_t = {d: hpool.tile([H, LN[d]], dt.float16, tag=f"h{d}",
                                 name=f"h{d}") for d in range(9)}

            warm_ps = pspool.tile([H, 2048], dt.float32, tag="ps", name="warm")
            for _ in range(8):
                nc.tensor.matmul(warm_ps[:, :512], lhsT=junk[:, :H], rhs=junk[:],
                                 start=True, stop=True)

            def jmm(psdst):
                """junk matmul gap-filler: keeps the PE HAM clock gate open.
                Writes a psum range that is dead or about to be start=True
                overwritten, so it never perturbs live data."""
                nc.tensor.matmul(psdst, lhsT=junk[:, :H], rhs=junk[:],
                                 start=True, stop=True, skip_group_check=True)

            def tanh_store(ps_ap, d, c0, w, name):
                """tanh psum chunk [level-d cols c0:c0+w] -> h_t[d]."""
                nc.scalar.activation(h_t[d][:, c0:c0 + w], ps_ap, AF.Tanh,
                                     bias=bh_t[:])

            def pairsum_piece(agg_ap, lo_ap, hi_ap):
                """contiguous fp16 add (DVE 2x_1p): evens half + odds half."""
                nc.vector.tensor_tensor(out=agg_ap, in0=lo_ap, in1=hi_ap,
                                        op=ALU.add)

            # ---- level 8: 4 psum chunks of 2048, each fed by 2 bands ----
            # h8 columns are locally de-interleaved per 2048-chunk (host
            # permutes the one-hot cols): evens half | odds half per 8 trees.
            agg7 = apool.tile([H, 4096], dt.float16, tag="agg7")
            egdr_ap = egdr[:].rearrange("p (two m) -> p two m", two=2)
            for c in range(4):
                ps = pspool.tile([H, 2048], dt.float32, tag="ps", name=f"ps8_{c}")
                if c >= 1:
                    jmm(ps[:, 0:512])
                    jmm(ps[:, 512:1024])
                for half in range(2):
                    band = l8_bands[2 * c + half]
                    po = half * 1024
                    if l8_dr:
                        rhs_all = band[:].rearrange("p (two n) -> p two n", two=2)
                        for j in range(2):
                            nc.tensor.matmul(
                                ps[:, po + j * 512:po + (j + 1) * 512],
                                lhsT=egdr_ap,
                                rhs=rhs_all[:, :, j * 512:(j + 1) * 512],
                                start=True, stop=True, perf_mode=PM.DoubleRow)
                    else:
                        for j in range(2):
                            nc.tensor.matmul(
                                ps[:, po + j * 512:po + (j + 1) * 512],
                                lhsT=eaug[:], rhs=band[:, j * 512:(j + 1) * 512],
                                start=True, stop=False)
                        for j in range(2):
                            nc.tensor.matmul(
                                ps[:, po + j * 512:po + (j + 1) * 512],
                                lhsT=gaug[:],
                                rhs=band[:, L8_BAND + j * 512:L8_BAND + (j + 1) * 512],
                                start=False, stop=True)
                tanh_store(ps[:], 8, c * 2048, 2048, f"t8_{c}")
                # agg7 piece for this h8 chunk: contiguous halves add
                pairsum_piece(agg7[:, c * 1024:(c + 1) * 1024],
                              h_t[8][:, c * 2048:c * 2048 + 1024],
                              h_t[8][:, c * 2048 + 1024:(c + 1) * 2048])

            # ---- level 7: 2 chunks of 2048 (de-interleaved psum layout) ----
            # Wh matmuls first (agg7 ready before x7 lands), E one-hots stop.
            x7bands = [x7a, x7b]
            for i in range(2):
                ps = pspool.tile([H, 2048], dt.float32, tag="ps", name=f"ps7_{i}")
                jmm(ps[:, 0:512])
                av = agg7[:, i * 2048:(i + 1) * 2048].rearrange(
                    "p (n two) -> p two n", two=2)
                for j in range(4):
                    par, jo = j // 2, (j % 2) * 512
                    nc.tensor.matmul(ps[:, j * 512:(j + 1) * 512], lhsT=wh[:],
                                     rhs=av[:, par:par + 1, jo:jo + 512],
                                     start=True, stop=False)
                for j in range(4):
                    nc.tensor.matmul(ps[:, j * 512:(j + 1) * 512], lhsT=eaug[:],
                                     rhs=x7bands[i][:, j * 512:(j + 1) * 512],
                                     start=False, stop=True)
                tanh_store(ps[:], 7, i * 2048, 2048, f"t7_{i}")

            # ---- level 6: one psum tile, two 1024-col groups so the first
            # tanh half fires right after h7 chunk 0 ----
            agg6 = apool.tile([H, 2048], dt.float16, tag="agg6")
            ps6 = pspool.tile([H, 2048], dt.float32, tag="ps", name="ps6")
            for g in range(2):
                pairsum_piece(agg6[:, g * 1024:(g + 1) * 1024],
                              h_t[7][:, g * 2048:g * 2048 + 1024],
                              h_t[7][:, g * 2048 + 1024:(g + 1) * 2048])
                if g == 0:
                    jmm(ps6[:, 0:512])
                for j in (2 * g, 2 * g + 1):
                    nc.tensor.matmul(ps6[:, j * 512:(j + 1) * 512], lhsT=wh[:],
                                     rhs=agg6[:, j * 512:(j + 1) * 512],
                                     start=True, stop=False)
                for j in (2 * g, 2 * g + 1):
                    nc.tensor.matmul(ps6[:, j * 512:(j + 1) * 512], lhsT=eaug[:],
                                     rhs=x6t[:, j * 512:(j + 1) * 512],
                                     start=False, stop=True)
                tanh_store(ps6[:, g * 1024:(g + 1) * 1024], 6, g * 1024, 1024,
                           f"t6_{g}")

            # ---- tail levels 5..0 share one psum tile; E prefilled ----
            # No DVE pairsum hop: each level is two stride-2 accumulating
            # matmuls (even/odd children) straight onto the E-prefilled psum.
            pt = pspool.tile([H, 2048], dt.float32, tag="ps", name="ps_tail")
            toff = {5: 0, 4: 1024, 3: 1536, 2: 1792, 1: 1920, 0: 1984}
            # bank-aligned prefills: L5 (banks 0-1), L4 (bank 2), L3..L0 fused
            # into one matmul (they share bank 3; start=True zeroes per-bank)
            for o, w in ((0, 512), (512, 512), (1024, 512), (1536, 480)):
                nc.tensor.matmul(pt[:, o:o + w], lhsT=eaug[:],
                                 rhs=smt[:, o:o + w],
                                 start=True, stop=False, skip_group_check=True)

            def sub_mms(d, o, hlo, w):
                """psum[o:o+w] += Wh^T evens + odds of h_{d+1}[hlo:hlo+2w]."""
                pv = h_t[d + 1][:, hlo:hlo + 2 * w].rearrange(
                    "p (n two) -> p two n", two=2)
                for par in range(2):
                    nc.tensor.matmul(pt[:, o:o + w], lhsT=wh[:],
                                     rhs=pv[:, par:par + 1, 0:w],
                                     start=False, stop=(par == 1),
                                     skip_group_check=True)

            # L5 in two 512 groups gated on the two t6 halves; junk fills the
            # tanh waits from dead psum (ps6 after t6 reads, pt after t5).
            sub_mms(5, 0, 0, 512)
            sub_mms(5, 512, 1024, 512)
            jmm(ps6[:, 0:512])
            tanh_store(pt[:, 0:512], 5, 0, 512, "t5a")
            tanh_store(pt[:, 512:1024], 5, 512, 512, "t5b")

            # Levels 4..0 run as two tree-group ladders (A: trees 0-15,
            # B: 16-31): group-B matmuls overlap group-A tanh. Gap fillers
            # live in ps6 (fully dead after t6): writing pt here would stall
            # the tanh/softmax reads on the coarse psum deps.
            jdead = [ps6[:, 1536:2048], ps6[:, 0:512], ps6[:, 512:1024],
                     ps6[:, 1024:1536], ps6[:, 0:1024]]
            for d in range(4, -1, -1):
                o, h = toff[d], LN[d] // 2
                sub_mms(d, o, 0, h)
                tanh_store(pt[:, o:o + h], d, 0, h, f"t{d}a")
                sub_mms(d, o + h, 2 * h, h)
                jmm(jdead[d])
                tanh_store(pt[:, o + h:o + 2 * h], d, h, h, f"t{d}b")

            # ---- logits + log_softmax (no ACT table switch) ----
            lg_ps = pt[0:TPC, 2016:2016 + NCLS]
            nc.tensor.matmul(lg_ps, lhsT=h_t[0][:, :TPC], rhs=wc[:],
                             start=True, stop=True, skip_group_check=True)
            for _ in range(4):
                jmm(ps6[:, 0:512])
                jmm(ps6[:, 512:1024])
            lg = smpool.tile([TPC, NCLS], dt.float32, tag="lg")
            nc.vector.tensor_tensor(out=lg[:], in0=lg_ps, in1=bc_rep[:], op=ALU.add)
            negmx = smpool.tile([TPC, 1], dt.float32, tag="negmx")
            nc.vector.tensor_reduce(negmx[:], lg[:], axis=AX.X, op=ALU.max,
                                    negate=True)
            ex = smpool.tile([TPC, NCLS], dt.float32, tag="ex")
            nc.scalar.activation(ex[:], lg[:], AF.Exp, bias=negmx[:])
            s = smpool.tile([TPC, 1], dt.float32, tag="s")
            nc.vector.tensor_reduce(s[:], ex[:], axis=AX.X, op=ALU.add)
            # ln(s), s in [1,6]: Horner with per-partition scalar s
            p = smpool.tile([TPC, 1], dt.float32, tag="p")
            nc.vector.tensor_scalar(out=p[:], in0=s[:], scalar1=float(LN_POLY[0]),
                                    scalar2=float(LN_POLY[1]), op0=ALU.mult,
                                    op1=ALU.add)
            for c in LN_POLY[2:]:
                nc.vector.tensor_scalar(out=p[:], in0=p[:], scalar1=s[:],
                                        scalar2=float(c), op0=ALU.mult, op1=ALU.add)
            # out = lg - (mx + ln s) = (lg - p) + negmx, fused in one op
            res = smpool.tile([TPC, NCLS], dt.float32, tag="res")
            nc.vector.tensor_scalar(out=res[:], in0=lg[:], scalar1=p[:],
                                    scalar2=negmx[:], op0=ALU.subtract,
                                    op1=ALU.add)
            nc.sync.dma_start(out=out_d[:], in_=res[:])

            if debug_taps:
                for d in range(9):
                    hd = nc.dram_tensor(f"h_dump{d}", [H, LN[d]], dt.float16,
                                        kind="ExternalOutput")
                    nc.scalar.dma_start(out=hd[:], in_=h_t[d][:])
                lgd = nc.dram_tensor("lg_dump", [TPC, NCLS], dt.float32,
                                     kind="ExternalOutput")
                nc.scalar.dma_start(out=lgd[:], in_=lg[:])

    nc.compile()
    return nc, "out"


def _get_program(l8_dr=L8_DR):
    key = bool(l8_dr)
    if key not in _PROGRAMS:
        _PROGRAMS[key] = _build_program(key)
    return _PROGRAMS[key]


def _host_inputs(node_type, E, Wh, bh, Wc, bc):
    """Per-core input maps: sharding + one-hot/index re-encoding (host side)."""
    FP8 = ml_dtypes.float8_e4m3
    nt = np.asarray(node_type).astype(np.int64).reshape(B, M)
    E = np.asarray(E, dtype=np.float32)
    Wh = np.asarray(Wh, dtype=np.float32)
    bh = np.asarray(bh, dtype=np.float32)
    Wc = np.asarray(Wc, dtype=np.float32)
    bc = np.asarray(bc, dtype=np.float32)

    G = np.tanh(E) @ Wh                      # leaf fold, [V, H]
    e_aug = E
    g_aug = G.astype(np.float32)
    eg_dr = np.concatenate([e_aug, g_aug], axis=1)   # [KV, 2H]: planes E|G

    shared = {
        "e_aug": e_aug.astype(np.float16),
        "g_aug": g_aug.astype(np.float16),
        "eg_dr": eg_dr.astype(FP8),
        "wh": Wh.astype(np.float16),
        "wc": Wc.astype(np.float16),
        "bc_rep": np.tile(bc.reshape(1, NCLS), (TPC, 1)).astype(np.float32),
        "bh": bh.reshape(H, 1).astype(np.float32),
    }

    in_maps = []
    for c in range(CORES):
        ntc = nt[c * TPC:(c + 1) * TPC]                  # [32, 1023]
        ohf = np.zeros((KV, TOTAL_COLS), dtype=np.float32)
        # L8 bands: x8 one-hots + summed leaf-pair one-hots, interleaved 2048s.
        # h8 cols are locally de-interleaved per 2048-chunk (8 trees): within
        # chunk, even-sibling nodes fill [0:1024), odd siblings [1024:2048).
        t8 = ntc[:, 255:511].ravel()                     # [8192]
        lt = ntc[:, 511:1023]
        a = lt[:, 0::2].ravel()
        b_ = lt[:, 1::2].ravel()
        p = np.arange(N8)
        t_, m_ = p // 256, p % 256
        pos8 = 2048 * (t_ // 8) + (t_ % 8) * 128 + m_ // 2 + (m_ % 2) * 1024
        xcol = 2 * L8_BAND * (pos8 // L8_BAND) + (pos8 % L8_BAND)
        ccol = xcol + L8_BAND
        ohf[t8, xcol] = 1.0
        np.add.at(ohf, (a, ccol), 1.0)
        np.add.at(ohf, (b_, ccol), 1.0)
        # x one-hots for levels 7..0 (level 7 de-interleaved per 2048-chunk
        # of 16 trees to match the h7 psum layout; 6..0 natural)
        for d in range(7, -1, -1):
            idx = ntc[:, (1 << d) - 1:(2 << d) - 1].ravel()
            k = np.arange(idx.size)
            if d == 7:
                t7, j7 = k // 128, k % 128
                k = 2048 * (t7 // 16) + (t7 % 16) * 64 + j7 // 2 \
                    + (j7 % 2) * 1024
            cols = OFF_X[d] + k
            ohf[idx, cols] = 1.0
        # repack into chunk-contiguous DRAM layout [N_CHUNKS, KV, CHUNK_W]
        oh3 = np.zeros((N_CHUNKS, KV, CHUNK_W), dtype=np.float32)
        for ci, (c0, w) in enumerate(CHUNK_FLAT):
            oh3[ci, :, :w] = ohf[:, c0:c0 + w]
        in_maps.append({"oh": oh3.astype(FP8), **shared})
    return in_maps


def kernel(node_type, parent_idx, depth, root_idx, E, Wh, bh, Wc, bc,
           _trace=False, _sim=False, _l8_dr=None):
    from concourse.bass_utils import run_bass_kernel_spmd

    l8_dr = L8_DR if _l8_dr is None else _l8_dr
    nc, out_name = _get_program(l8_dr)
    in_maps = _host_inputs(node_type, E, Wh, bh, Wc, bc)

    if _sim:
        from concourse.bass_interp import CoreSim
        outs = []
        ncores = _sim if isinstance(_sim, int) and _sim > 1 else CORES
        for m in in_maps[:ncores]:
            sim = CoreSim(nc, trace=False)
            for k, v in m.items():
                sim.tensor(k)[:] = v
            sim.simulate(check_with_hw=False)
            outs.append(np.array(sim.tensor(out_name)))
        return np.concatenate(outs, axis=0).astype(np.float32)

    results = run_bass_kernel_spmd(
        nc, in_maps, core_ids=list(range(CORES)), trace=_trace,
    )
    out = np.concatenate([r[out_name] for r in results.results], axis=0)
    out = np.ascontiguousarray(out).astype(np.float32)
    if _trace:
        return out, results
    return out

